# revision 16
# baseline (speedup 1.0000x reference)
"""Trainium2 Bass kernel for nn_ModalityConsisLoss (8 NeuronCores, data-parallel).

Reference computation:
    v_spa/v_seq = concat([f[:,a,:], f[:,2,:]], -1) @ W + b   for a in (0,1,3)  -> [3B, D]
    z = normalize_rows(concat([v_spa, v_seq]))               -> [6B, D]
    sim = z @ z.T ;  pos = diag pairs (i, i+3B)
    loss = sum(-pos/T) + sum(log(rowsum(exp(sim/T)) - diag)) / (6B)

Strategy (data-parallel over B):
  Each core owns B/8 = 256 batch rows -> 1536 of the 12288 z-rows
  (rows of both modalities for its batch slice, so pos pairs stay local).
  Per core, on device, per modality half (spa then seq):
    - load f shard, PE-transpose -> fT, projection matmuls -> vT half
    - column norms via ones-matmul; r = 16 * rsqrt(ssq) via exp/ln
    - zT_half = fp8_e4m3(vT * r)  [512, 768]  (x16 scaling keeps fp8 in
      normal range; folded back via the exp() scale and the pos term)
    - AllGather the half (so the spa gather overlaps the seq prologue,
      and the seq gather overlaps the first sim tiles)
  sim tiles: DoubleRow fp8 matmuls (K=256 per instruction) of
  zT_local.T @ zT_all with fused exp(sim/(T*256)) + row-sum on ACT.
  denom = rowsum - e^2 ; partial loss = sum(log denom) - (2/T)*sum(pos).
  Host sums the 8 partial scalars (the trivial all-reduce of the loss).
"""
import sys
from contextlib import ExitStack

sys.path.insert(0, "/opt/trn_rl_repo")

import numpy as np

import concourse.bass as bass
import concourse.mybir as mybir
import concourse.tile as tile
from concourse import bacc
from concourse import bass_utils
from concourse.masks import make_identity

F32 = mybir.dt.float32
BF16 = mybir.dt.bfloat16
FP8 = mybir.dt.float8e4
AF = mybir.ActivationFunctionType
ALU = mybir.AluOpType
DR = mybir.MatmulPerfMode.DoubleRow

N_CORES = 8
B = 2048
BL = B // N_CORES          # 256 local batch rows
D = 512
KB = D // 128              # 4 d blocks of 128
HROWS = 3 * BL             # 768 rows per modality half
LROWS = 2 * HROWS          # 1536 local z-rows (spa 768 | seq 768)
R = N_CORES * LROWS        # 12288 total rows
HALL = N_CORES * HROWS     # 6144 gathered columns per half
IB = LROWS // 128          # 12 row blocks of 128 per core
SIMW = 1536                # sim chunk width (3 PSUM banks, one ACT op)
CC = HALL // SIMW          # 3 sim column chunks per half
LH = (0, 1, 3)             # left heads of the pairs (x, 2)
TEMP = 0.5
ZSCALE = 16.0              # fp8 z scaling
ESCALE = (1.0 / TEMP) / (ZSCALE * ZSCALE)
POS_COEF = (-2.0 / TEMP) / (ZSCALE * ZSCALE)
E2 = float(np.exp(2.0))    # diagonal term exp(2 * ||z||^2), ||z|| == 1
INV_COUNT = 1.0 / R        # final 1/(2*half)


def _body(ctx, nc, tc, f_aps, w_ap, b_ap, out_ap):
    const_pool = ctx.enter_context(tc.tile_pool(name="const", bufs=1))
    small_pool = ctx.enter_context(tc.tile_pool(name="small", bufs=1))
    vt_pool = ctx.enter_context(tc.tile_pool(name="vt", bufs=1))
    dram_pool = ctx.enter_context(tc.tile_pool(name="dram", bufs=1,
                                               space="DRAM"))
    big_pool = ctx.enter_context(tc.tile_pool(name="big", bufs=1))

    ident = const_pool.tile([128, 128], F32)
    make_identity(nc, ident[:])
    ones_col = const_pool.tile([128, 1], F32)
    nc.vector.memset(ones_col[:], 1.0)
    ones_row = const_pool.tile([1, 128], F32)
    nc.vector.memset(ones_row[:], 1.0)
    neg_e2 = const_pool.tile([128, 1], F32)
    nc.vector.memset(neg_e2[:], -E2)
    ln_zs = const_pool.tile([1, 1], F32)
    nc.vector.memset(ln_zs[:], float(np.log(ZSCALE)))

    # b columns: [128, 4] (per d_out block)
    b_col = const_pool.tile([128, 4], F32)
    for m in range(KB):
        nc.sync.dma_start(b_col[:, m:m + 1], b_ap[m * 128:(m + 1) * 128])

    w_bf = const_pool.tile([128, 8, D], BF16)

    vT = vt_pool.tile([128, KB, LROWS], F32)       # [d_out(blk,128), rows]
    zT_loc = small_pool.tile([128, KB, LROWS], FP8, tag="zT_loc")
    r_row = small_pool.tile([1, LROWS], F32, tag="r_row")
    zT_all = [None, None]

    with tc.tile_pool(name="fstage", bufs=4) as fst_pool, \
         tc.tile_pool(name="ftrans", bufs=1) as ft_pool, \
         tc.tile_pool(name="sq", bufs=2) as sq_pool, \
         tc.tile_pool(name="ps_t", bufs=2, space="PSUM") as ps_t, \
         tc.tile_pool(name="ps_proj", bufs=2, space="PSUM") as ps_proj, \
         tc.tile_pool(name="ps_s", bufs=2, space="PSUM") as ps_s:

        # f loads first: the transposes (start of the PE critical path)
        # need them; W can land while the first transposes run.
        f_sts = {}
        for mod in range(2):
            for h in range(2):
                f_st = fst_pool.tile([128, 4 * D], F32,
                                     name=f"f_st{mod}{h}", tag="f_st")
                nc.sync.dma_start(
                    f_st[:], f_aps[mod][h * 128:(h + 1) * 128, :, :])
                f_sts[(mod, h)] = f_st

        # W: [1024, 512] f32 -> bf16 [128, 8(kblk), 512(d_out)]
        w_st = fst_pool.tile([128, 8, D], F32, tag="w_st", bufs=1)
        for kb in range(8):
            nc.sync.dma_start(w_st[:, kb, :], w_ap[kb * 128:(kb + 1) * 128, :])
        nc.vector.tensor_copy(w_bf[:], w_st[:])

        for mod in range(2):                   # 0 = spa, 1 = seq
            c0 = mod * HROWS
            # ---- transpose f ----
            fT = ft_pool.tile([128, 4, KB, 2 * 128], BF16, name=f"fT{mod}",
                              tag=f"fT{mod}")
            for h in range(2):                 # halves of 256 local rows
                f_st = f_sts[(mod, h)]
                for a in range(4):
                    for kb in range(KB):
                        pst = ps_t.tile([128, 128], F32, name="pst", tag="pst")
                        nc.tensor.transpose(
                            pst[:],
                            f_st[:, a * D + kb * 128: a * D + (kb + 1) * 128],
                            ident[:])
                        nc.vector.tensor_copy(
                            fT[:, a, kb, h * 128:(h + 1) * 128], pst[:])
            # ---- projection ----
            for pa in range(3):
                for m in range(KB):
                    psv = ps_proj.tile([128, 2 * 128], F32, name="psv",
                                       tag="psv")
                    for kk in range(8):
                        head = LH[pa] if kk < 4 else 2
                        kb = kk % 4
                        nc.tensor.matmul(
                            psv[:],
                            lhsT=w_bf[:, kk, m * 128:(m + 1) * 128],
                            rhs=fT[:, head, kb, :],
                            start=(kk == 0), stop=(kk == 7))
                    col0 = c0 + pa * 256
                    nc.vector.tensor_scalar_add(
                        vT[:, m, col0:col0 + 256], psv[:], b_col[:, m:m + 1])

            # ---- norms: ssq over d for this half's 768 columns ----
            ssq = small_pool.tile([1, HROWS], F32, name=f"ssq{mod}",
                                  tag=f"ssq{mod}")
            for co, cw in ((0, 512), (512, 256)):
                ps_ssq = ps_s.tile([1, 512], F32, name="ps_ssq", tag="ps_s")
                for m in range(KB):
                    sq = sq_pool.tile([128, 512], F32, name="sq", tag="sq")
                    nc.vector.tensor_mul(sq[:, :cw],
                                         vT[:, m, c0 + co:c0 + co + cw],
                                         vT[:, m, c0 + co:c0 + co + cw])
                    nc.tensor.matmul(ps_ssq[:, :cw], lhsT=ones_col[:],
                                     rhs=sq[:, :cw],
                                     start=(m == 0), stop=(m == KB - 1))
                nc.vector.tensor_copy(ssq[:, co:co + cw], ps_ssq[:, :cw])

            # r = ZSCALE / sqrt(ssq) = exp(-0.5*ln(ssq) + ln(ZSCALE))
            lnss = small_pool.tile([1, HROWS], F32, name=f"lnss{mod}",
                                   tag=f"lnss{mod}")
            nc.scalar.activation(lnss[:], ssq[:], AF.Ln)
            nc.scalar.activation(r_row[:, c0:c0 + HROWS], lnss[:], AF.Exp,
                                 scale=-0.5, bias=ln_zs[:])

            # zT_loc half = fp8(vT * r)
            for co, cw in ((0, 512), (512, 256)):
                rb = ps_s.tile([128, 512], F32, name="rb", tag="rb")
                nc.tensor.matmul(rb[:, :cw], lhsT=ones_row[:],
                                 rhs=r_row[:, c0 + co:c0 + co + cw],
                                 start=True, stop=True)
                for m in range(KB):
                    nc.vector.tensor_mul(
                        zT_loc[:, m, c0 + co:c0 + co + cw],
                        vT[:, m, c0 + co:c0 + co + cw], rb[:, :cw])

            # ---- AllGather this half ----
            ag_in = dram_pool.tile([4 * 128, HROWS], FP8, name=f"ag_in{mod}",
                                   tag=f"ag_in{mod}")
            ag_out = dram_pool.tile([N_CORES * 4 * 128, HROWS], FP8,
                                    addr_space="Shared", name=f"ag_out{mod}",
                                    tag=f"ag_out{mod}")
            for m in range(KB):
                nc.sync.dma_start(ag_in[m * 128:(m + 1) * 128, :],
                                  zT_loc[:, m, c0:c0 + HROWS])
            nc.gpsimd.collective_compute(
                "AllGather", ALU.bypass,
                replica_groups=[list(range(N_CORES))],
                ins=[ag_in.opt()], outs=[ag_out.opt()])
            zT_all[mod] = big_pool.tile([128, KB, HALL], FP8,
                                        name=f"zT_all{mod}", tag=f"zTa{mod}")
            for rr in range(N_CORES):
                for m in range(KB):
                    nc.sync.dma_start(
                        zT_all[mod][:, m, rr * HROWS:(rr + 1) * HROWS],
                        ag_out[rr * 512 + m * 128: rr * 512 + (m + 1) * 128, :])

        # ---- pos_i = r_i * r_{i+768} * sum_d vT[d, i] * vT[d, i+768] ----
        pos_raw = small_pool.tile([1, HROWS], F32, tag="pos_raw")
        for co, cw in ((0, 512), (512, 256)):
            ps_pp = ps_s.tile([1, 512], F32, name="ps_pp", tag="ps_s")
            for m in range(KB):
                pp = sq_pool.tile([128, 512], F32, name="pp", tag="sq")
                nc.vector.tensor_mul(pp[:, :cw], vT[:, m, co:co + cw],
                                     vT[:, m, HROWS + co:HROWS + co + cw])
                nc.tensor.matmul(ps_pp[:, :cw], lhsT=ones_col[:],
                                 rhs=pp[:, :cw],
                                 start=(m == 0), stop=(m == KB - 1))
            nc.vector.tensor_copy(pos_raw[:, co:co + cw], ps_pp[:, :cw])
        rrp = small_pool.tile([1, HROWS], F32, tag="rrp")
        nc.vector.tensor_mul(rrp[:], r_row[:, 0:HROWS], r_row[:, HROWS:LROWS])
        pos_row = small_pool.tile([1, HROWS], F32, tag="pos_row")
        nc.vector.tensor_mul(pos_row[:], pos_raw[:], rrp[:])
        pos_sum = small_pool.tile([1, 1], F32, tag="pos_sum")
        nc.vector.tensor_reduce(pos_sum[:], pos_row[:],
                                axis=mybir.AxisListType.X, op=ALU.add)

    # ---------- sim tiles + fused exp/rowsum (DoubleRow fp8) ----------
    # stats col layout: [ib][mod * CC + cc] so the per-ib reduce is a
    # contiguous innermost group of 2*CC.
    stats = small_pool.tile([128, 2 * IB * CC], F32, tag="stats")
    with tc.tile_pool(name="ps_sim", bufs=2, space="PSUM") as ps_sim:
        for mod in range(2):
            for ib in range(IB):
                for cc in range(CC):
                    ps = ps_sim.tile([128, SIMW], F32, name="ps_sim",
                                     tag="ps_sim")
                    for jt in range(SIMW // 512):
                        j0 = cc * SIMW + jt * 512
                        for g in range(2):
                            nc.tensor.matmul(
                                ps[:, jt * 512:(jt + 1) * 512],
                                lhsT=zT_loc[:, 2 * g:2 * g + 2,
                                            ib * 128:(ib + 1) * 128],
                                rhs=zT_all[mod][:, 2 * g:2 * g + 2,
                                                j0:j0 + 512],
                                start=(g == 0), stop=(g == 1),
                                perf_mode=DR)
                    scol = ib * 2 * CC + mod * CC + cc
                    nc.scalar.activation(
                        ps[:], ps[:], AF.Exp, scale=ESCALE,
                        accum_out=stats[:, scol:scol + 1])

    # ---------- final reduction ----------
    with tc.tile_pool(name="ps_fin", bufs=1, space="PSUM") as ps_fin:
        denom = small_pool.tile([128, IB], F32, tag="denom")
        nc.vector.tensor_reduce(
            denom[:], stats.rearrange("p (i x) -> p i x", x=2 * CC),
            axis=mybir.AxisListType.X, op=ALU.add)
        logd = small_pool.tile([128, IB], F32, tag="logd")
        nc.scalar.activation(logd[:], denom[:], AF.Ln, bias=neg_e2[:])
        logsum = small_pool.tile([128, 1], F32, tag="logsum")
        nc.vector.tensor_reduce(logsum[:], logd[:],
                                axis=mybir.AxisListType.X, op=ALU.add)
        fin = ps_fin.tile([1, 1], F32, tag="fin")
        nc.tensor.matmul(fin[:], lhsT=ones_col[:], rhs=logsum[:],
                         start=True, stop=True)
        res = small_pool.tile([1, 1], F32, tag="res")
        # res = (pos_sum * POS_COEF + sum(log denom)) / R
        nc.vector.scalar_tensor_tensor(res[:], pos_sum[:], POS_COEF,
                                       fin[:], op0=ALU.mult, op1=ALU.add)
        nc.vector.tensor_scalar_mul(res[:], res[:], INV_COUNT)
        nc.sync.dma_start(out_ap[:], res[:])


_NC_CACHE = None


def build_nc():
    global _NC_CACHE
    if _NC_CACHE is not None:
        return _NC_CACHE
    nc = bacc.Bacc("TRN2", target_bir_lowering=False, debug=False,
                   num_devices=N_CORES)
    f_spa = nc.dram_tensor("f_spa", [BL, 4, D], F32, kind="ExternalInput").ap()
    f_seq = nc.dram_tensor("f_seq", [BL, 4, D], F32, kind="ExternalInput").ap()
    w_ap = nc.dram_tensor("W", [2 * D, D], F32, kind="ExternalInput").ap()
    b_ap = nc.dram_tensor("b", [D], F32, kind="ExternalInput").ap()
    out_ap = nc.dram_tensor("out", [1, 1], F32, kind="ExternalOutput").ap()
    with tile.TileContext(nc) as tc, ExitStack() as ctx:
        _body(ctx, nc, tc, (f_spa, f_seq), w_ap, b_ap, out_ap)
    nc.compile()
    _NC_CACHE = nc
    return nc


def run(inputs, **kw):
    nc = build_nc()
    f_seq = np.ascontiguousarray(np.asarray(inputs["f_seq"], dtype=np.float32))
    f_spa = np.ascontiguousarray(np.asarray(inputs["f_spa"], dtype=np.float32))
    W = np.ascontiguousarray(np.asarray(inputs["W"], dtype=np.float32))
    b = np.ascontiguousarray(np.asarray(inputs["b"], dtype=np.float32))
    in_maps = []
    for c in range(N_CORES):
        sl = slice(c * BL, (c + 1) * BL)
        in_maps.append({"f_seq": np.ascontiguousarray(f_seq[sl]),
                        "f_spa": np.ascontiguousarray(f_spa[sl]),
                        "W": W, "b": b})
    res = bass_utils.run_bass_kernel_spmd(
        nc, in_maps, core_ids=list(range(N_CORES)), **kw)
    total = np.float64(0.0)
    for c in range(N_CORES):
        total += np.float64(res.results[c]["out"][0, 0])
    return np.float32(total), res


def kernel(**inputs) -> np.ndarray:
    loss, _ = run(inputs)
    return np.asarray(loss, dtype=np.float32)


if __name__ == "__main__":
    rng = np.random.default_rng(0)
    inputs = {
        "f_seq": rng.standard_normal((B, 4, D), dtype=np.float32),
        "f_spa": rng.standard_normal((B, 4, D), dtype=np.float32),
        "W": (rng.standard_normal((2 * D, D), dtype=np.float32) * 0.02),
        "b": np.zeros((D,), dtype=np.float32),
    }
    print(kernel(**inputs))


# revision 22
# speedup vs baseline: 1.0071x; 1.0071x over previous
"""Trainium2 Bass kernel for nn_ModalityConsisLoss (8 NeuronCores, data-parallel).

Reference computation:
    v_spa/v_seq = concat([f[:,a,:], f[:,2,:]], -1) @ W + b   for a in (0,1,3)  -> [3B, D]
    z = normalize_rows(concat([v_spa, v_seq]))               -> [6B, D]
    sim = z @ z.T ;  pos = diag pairs (i, i+3B)
    loss = sum(-pos/T) + sum(log(rowsum(exp(sim/T)) - diag)) / (6B)

Strategy (data-parallel over B):
  Each core owns B/8 = 256 batch rows -> 1536 of the 12288 z-rows
  (rows of both modalities for its batch slice, so pos pairs stay local).
  Per core, on device, per modality half (spa then seq):
    - load f shard, PE-transpose -> fT, projection matmuls -> vT half
    - column norms via ones-matmul; r = 16 * rsqrt(ssq) via exp/ln
    - zT_half = fp8_e4m3(vT * r)  [512, 768]  (x16 scaling keeps fp8 in
      normal range; folded back via the exp() scale and the pos term)
    - AllGather the half (so the spa gather overlaps the seq prologue,
      and the seq gather overlaps the first sim tiles)
  sim tiles: DoubleRow fp8 matmuls (K=256 per instruction) of
  zT_local.T @ zT_all with fused exp(sim/(T*256)) + row-sum on ACT.
  denom = rowsum - e^2 ; partial loss = sum(log denom) - (2/T)*sum(pos).
  Host sums the 8 partial scalars (the trivial all-reduce of the loss).
"""
import sys
from contextlib import ExitStack

sys.path.insert(0, "/opt/trn_rl_repo")

import numpy as np

import concourse.bass as bass
import concourse.mybir as mybir
import concourse.tile as tile
from concourse import bacc
from concourse import bass_utils
from concourse.masks import make_identity

F32 = mybir.dt.float32
BF16 = mybir.dt.bfloat16
FP8 = mybir.dt.float8e4
AF = mybir.ActivationFunctionType
ALU = mybir.AluOpType
DR = mybir.MatmulPerfMode.DoubleRow

N_CORES = 8
B = 2048
BL = B // N_CORES          # 256 local batch rows
D = 512
KB = D // 128              # 4 d blocks of 128
HROWS = 3 * BL             # 768 rows per modality half
LROWS = 2 * HROWS          # 1536 local z-rows (spa 768 | seq 768)
R = N_CORES * LROWS        # 12288 total rows
HALL = N_CORES * HROWS     # 6144 gathered columns per half
IB = LROWS // 128          # 12 row blocks of 128 per core
SIMW = 1536                # sim chunk width (3 PSUM banks, one ACT op)
CC = HALL // SIMW          # 3 sim column chunks per half
LH = (0, 1, 3)             # left heads of the pairs (x, 2)
TEMP = 0.5
ZSCALE = 16.0              # fp8 z scaling
ESCALE = (1.0 / TEMP) / (ZSCALE * ZSCALE)
POS_COEF = (-2.0 / TEMP) / (ZSCALE * ZSCALE)
E2 = float(np.exp(2.0))    # diagonal term exp(2 * ||z||^2), ||z|| == 1
INV_COUNT = 1.0 / R        # final 1/(2*half)


def _body(ctx, nc, tc, f_aps, w_ap, b_ap, out_ap):
    const_pool = ctx.enter_context(tc.tile_pool(name="const", bufs=1))
    small_pool = ctx.enter_context(tc.tile_pool(name="small", bufs=1))
    vt_pool = ctx.enter_context(tc.tile_pool(name="vt", bufs=1))
    dram_pool = ctx.enter_context(tc.tile_pool(name="dram", bufs=1,
                                               space="DRAM"))
    big_pool = ctx.enter_context(tc.tile_pool(name="big", bufs=1))

    ident = const_pool.tile([128, 128], F32)
    make_identity(nc, ident[:])
    ones_col = const_pool.tile([128, 1], F32)
    nc.vector.memset(ones_col[:], 1.0)
    ones_row = const_pool.tile([1, 128], F32)
    nc.vector.memset(ones_row[:], 1.0)
    neg_e2 = const_pool.tile([128, 1], F32)
    nc.vector.memset(neg_e2[:], -E2)
    ln_zs = const_pool.tile([1, 1], F32)
    nc.vector.memset(ln_zs[:], float(np.log(ZSCALE)))

    # b columns: [128, 4] (per d_out block)
    b_col = const_pool.tile([128, 4], F32)
    for m in range(KB):
        nc.sync.dma_start(b_col[:, m:m + 1], b_ap[m * 128:(m + 1) * 128])
    w_bf = const_pool.tile([128, 8, D], BF16)

    vT = vt_pool.tile([128, KB, LROWS], F32)       # [d_out(blk,128), rows]
    zT_loc = small_pool.tile([128, KB, LROWS], FP8, tag="zT_loc")
    r_row = small_pool.tile([1, LROWS], F32, tag="r_row")
    zT_all = [None, None]

    with tc.tile_pool(name="fstage", bufs=4) as fst_pool, \
         tc.tile_pool(name="ftrans", bufs=1) as ft_pool, \
         tc.tile_pool(name="sq", bufs=2) as sq_pool, \
         tc.tile_pool(name="ps_t", bufs=2, space="PSUM") as ps_t, \
         tc.tile_pool(name="ps_proj", bufs=2, space="PSUM") as ps_proj, \
         tc.tile_pool(name="ps_s", bufs=2, space="PSUM") as ps_s:

        # f loads first: the transposes (start of the PE critical path)
        # need them; W can land while the first transposes run.
        f_sts = {}
        for mod in range(2):
            for h in range(2):
                f_st = fst_pool.tile([128, 4 * D], F32,
                                     name=f"f_st{mod}{h}", tag="f_st")
                nc.sync.dma_start(
                    f_st[:], f_aps[mod][h * 128:(h + 1) * 128, :, :])
                f_sts[(mod, h)] = f_st

        # W: [1024, 512] f32 -> bf16 [128, 8(kblk), 512(d_out)]
        w_st = fst_pool.tile([128, 8, D], F32, tag="w_st", bufs=1)
        for kb in range(8):
            nc.sync.dma_start(w_st[:, kb, :], w_ap[kb * 128:(kb + 1) * 128, :])
        nc.vector.tensor_copy(w_bf[:], w_st[:])

        for mod in range(2):                   # 0 = spa, 1 = seq
            c0 = mod * HROWS
            # ---- transpose f ----
            fT = ft_pool.tile([128, 4, KB, 2 * 128], BF16, name=f"fT{mod}",
                              tag=f"fT{mod}")
            for h in range(2):                 # halves of 256 local rows
                f_st = f_sts[(mod, h)]
                for a in range(4):
                    for kb in range(KB):
                        pst = ps_t.tile([128, 128], F32, name="pst", tag="pst")
                        nc.tensor.transpose(
                            pst[:],
                            f_st[:, a * D + kb * 128: a * D + (kb + 1) * 128],
                            ident[:])
                        nc.vector.tensor_copy(
                            fT[:, a, kb, h * 128:(h + 1) * 128], pst[:])
            # ---- projection ----
            for pa in range(3):
                for m in range(KB):
                    psv = ps_proj.tile([128, 2 * 128], F32, name="psv",
                                       tag="psv")
                    for kk in range(8):
                        head = LH[pa] if kk < 4 else 2
                        kb = kk % 4
                        nc.tensor.matmul(
                            psv[:],
                            lhsT=w_bf[:, kk, m * 128:(m + 1) * 128],
                            rhs=fT[:, head, kb, :],
                            start=(kk == 0), stop=(kk == 7))
                    col0 = c0 + pa * 256
                    nc.vector.tensor_scalar_add(
                        vT[:, m, col0:col0 + 256], psv[:], b_col[:, m:m + 1])

            # ---- norms: ssq over d for this half's 768 columns ----
            ssq = small_pool.tile([1, HROWS], F32, name=f"ssq{mod}",
                                  tag=f"ssq{mod}")
            for co, cw in ((0, 512), (512, 256)):
                ps_ssq = ps_s.tile([1, 512], F32, name="ps_ssq", tag="ps_s")
                for m in range(KB):
                    sq = sq_pool.tile([128, 512], F32, name="sq", tag="sq")
                    nc.vector.tensor_mul(sq[:, :cw],
                                         vT[:, m, c0 + co:c0 + co + cw],
                                         vT[:, m, c0 + co:c0 + co + cw])
                    nc.tensor.matmul(ps_ssq[:, :cw], lhsT=ones_col[:],
                                     rhs=sq[:, :cw],
                                     start=(m == 0), stop=(m == KB - 1))
                nc.vector.tensor_copy(ssq[:, co:co + cw], ps_ssq[:, :cw])

            # r = ZSCALE / sqrt(ssq) = exp(-0.5*ln(ssq) + ln(ZSCALE))
            lnss = small_pool.tile([1, HROWS], F32, name=f"lnss{mod}",
                                   tag=f"lnss{mod}")
            nc.scalar.activation(lnss[:], ssq[:], AF.Ln)
            nc.scalar.activation(r_row[:, c0:c0 + HROWS], lnss[:], AF.Exp,
                                 scale=-0.5, bias=ln_zs[:])

            # zT_loc half = fp8(vT * r)
            for co, cw in ((0, 512), (512, 256)):
                rb = ps_s.tile([128, 512], F32, name="rb", tag="rb")
                nc.tensor.matmul(rb[:, :cw], lhsT=ones_row[:],
                                 rhs=r_row[:, c0 + co:c0 + co + cw],
                                 start=True, stop=True)
                for m in range(KB):
                    nc.vector.tensor_mul(
                        zT_loc[:, m, c0 + co:c0 + co + cw],
                        vT[:, m, c0 + co:c0 + co + cw], rb[:, :cw])

            # ---- AllGather this half ----
            ag_in = dram_pool.tile([4 * 128, HROWS], FP8, name=f"ag_in{mod}",
                                   tag=f"ag_in{mod}")
            ag_out = dram_pool.tile([N_CORES * 4 * 128, HROWS], FP8,
                                    addr_space="Shared", name=f"ag_out{mod}",
                                    tag=f"ag_out{mod}")
            for m in range(KB):
                nc.sync.dma_start(ag_in[m * 128:(m + 1) * 128, :],
                                  zT_loc[:, m, c0:c0 + HROWS])
            nc.gpsimd.collective_compute(
                "AllGather", ALU.bypass,
                replica_groups=[list(range(N_CORES))],
                ins=[ag_in.opt()], outs=[ag_out.opt()])
            zT_all[mod] = big_pool.tile([128, KB, HALL], FP8,
                                        name=f"zT_all{mod}", tag=f"zTa{mod}")
            for rr in range(N_CORES):
                for m in range(KB):
                    nc.sync.dma_start(
                        zT_all[mod][:, m, rr * HROWS:(rr + 1) * HROWS],
                        ag_out[rr * 512 + m * 128: rr * 512 + (m + 1) * 128, :])

        # ---- pos_i = r_i * r_{i+768} * sum_d vT[d, i] * vT[d, i+768] ----
        pos_raw = small_pool.tile([1, HROWS], F32, tag="pos_raw")
        for co, cw in ((0, 512), (512, 256)):
            ps_pp = ps_s.tile([1, 512], F32, name="ps_pp", tag="ps_s")
            for m in range(KB):
                pp = sq_pool.tile([128, 512], F32, name="pp", tag="sq")
                nc.vector.tensor_mul(pp[:, :cw], vT[:, m, co:co + cw],
                                     vT[:, m, HROWS + co:HROWS + co + cw])
                nc.tensor.matmul(ps_pp[:, :cw], lhsT=ones_col[:],
                                 rhs=pp[:, :cw],
                                 start=(m == 0), stop=(m == KB - 1))
            nc.vector.tensor_copy(pos_raw[:, co:co + cw], ps_pp[:, :cw])
        rrp = small_pool.tile([1, HROWS], F32, tag="rrp")
        nc.vector.tensor_mul(rrp[:], r_row[:, 0:HROWS], r_row[:, HROWS:LROWS])
        pos_row = small_pool.tile([1, HROWS], F32, tag="pos_row")
        nc.vector.tensor_mul(pos_row[:], pos_raw[:], rrp[:])
        pos_sum = small_pool.tile([1, 1], F32, tag="pos_sum")
        nc.vector.tensor_reduce(pos_sum[:], pos_row[:],
                                axis=mybir.AxisListType.X, op=ALU.add)

    # ---------- sim tiles + fused exp/rowsum (DoubleRow fp8) ----------
    # stats col layout: [ib][mod * CC + cc] so the per-ib reduce is a
    # contiguous innermost group of 2*CC.
    stats = small_pool.tile([128, 2 * IB * CC], F32, tag="stats")
    with tc.tile_pool(name="ps_sim", bufs=2, space="PSUM") as ps_sim:
        for mod in range(2):
            for ib in range(IB):
                for cc in range(CC):
                    ps = ps_sim.tile([128, SIMW], F32, name="ps_sim",
                                     tag="ps_sim")
                    for jt in range(SIMW // 512):
                        j0 = cc * SIMW + jt * 512
                        for g in range(2):
                            nc.tensor.matmul(
                                ps[:, jt * 512:(jt + 1) * 512],
                                lhsT=zT_loc[:, 2 * g:2 * g + 2,
                                            ib * 128:(ib + 1) * 128],
                                rhs=zT_all[mod][:, 2 * g:2 * g + 2,
                                                j0:j0 + 512],
                                start=(g == 0), stop=(g == 1),
                                perf_mode=DR)
                    scol = ib * 2 * CC + mod * CC + cc
                    nc.scalar.activation(
                        ps[:], ps[:], AF.Exp, scale=ESCALE,
                        accum_out=stats[:, scol:scol + 1])

    # ---------- final reduction ----------
    with tc.tile_pool(name="ps_fin", bufs=1, space="PSUM") as ps_fin:
        denom = small_pool.tile([128, IB], F32, tag="denom")
        nc.vector.tensor_reduce(
            denom[:], stats.rearrange("p (i x) -> p i x", x=2 * CC),
            axis=mybir.AxisListType.X, op=ALU.add)
        logd = small_pool.tile([128, IB], F32, tag="logd")
        nc.scalar.activation(logd[:], denom[:], AF.Ln, bias=neg_e2[:])
        logsum = small_pool.tile([128, 1], F32, tag="logsum")
        nc.vector.tensor_reduce(logsum[:], logd[:],
                                axis=mybir.AxisListType.X, op=ALU.add)
        fin = ps_fin.tile([1, 1], F32, tag="fin")
        nc.tensor.matmul(fin[:], lhsT=ones_col[:], rhs=logsum[:],
                         start=True, stop=True)
        res = small_pool.tile([1, 1], F32, tag="res")
        # res = (pos_sum * POS_COEF + sum(log denom)) / R
        nc.vector.scalar_tensor_tensor(res[:], pos_sum[:], POS_COEF,
                                       fin[:], op0=ALU.mult, op1=ALU.add)
        nc.vector.tensor_scalar_mul(res[:], res[:], INV_COUNT)
        nc.sync.dma_start(out_ap[:], res[:])


_NC_CACHE = None


def build_nc():
    global _NC_CACHE
    if _NC_CACHE is not None:
        return _NC_CACHE
    nc = bacc.Bacc("TRN2", target_bir_lowering=False, debug=False,
                   num_devices=N_CORES)
    f_spa = nc.dram_tensor("f_spa", [BL, 4, D], F32, kind="ExternalInput").ap()
    f_seq = nc.dram_tensor("f_seq", [BL, 4, D], F32, kind="ExternalInput").ap()
    w_ap = nc.dram_tensor("W", [2 * D, D], F32, kind="ExternalInput").ap()
    b_ap = nc.dram_tensor("b", [D], F32, kind="ExternalInput").ap()
    out_ap = nc.dram_tensor("out", [1, 1], F32, kind="ExternalOutput").ap()
    with tile.TileContext(nc) as tc, ExitStack() as ctx:
        _body(ctx, nc, tc, (f_spa, f_seq), w_ap, b_ap, out_ap)
    nc.compile()
    _NC_CACHE = nc
    return nc


def run(inputs, **kw):
    nc = build_nc()
    f_seq = np.ascontiguousarray(np.asarray(inputs["f_seq"], dtype=np.float32))
    f_spa = np.ascontiguousarray(np.asarray(inputs["f_spa"], dtype=np.float32))
    W = np.ascontiguousarray(np.asarray(inputs["W"], dtype=np.float32))
    b = np.ascontiguousarray(np.asarray(inputs["b"], dtype=np.float32))
    in_maps = []
    for c in range(N_CORES):
        sl = slice(c * BL, (c + 1) * BL)
        in_maps.append({"f_seq": np.ascontiguousarray(f_seq[sl]),
                        "f_spa": np.ascontiguousarray(f_spa[sl]),
                        "W": W, "b": b})
    res = bass_utils.run_bass_kernel_spmd(
        nc, in_maps, core_ids=list(range(N_CORES)), **kw)
    total = np.float64(0.0)
    for c in range(N_CORES):
        total += np.float64(res.results[c]["out"][0, 0])
    return np.float32(total), res


def kernel(**inputs) -> np.ndarray:
    loss, _ = run(inputs)
    return np.asarray(loss, dtype=np.float32)


if __name__ == "__main__":
    rng = np.random.default_rng(0)
    inputs = {
        "f_seq": rng.standard_normal((B, 4, D), dtype=np.float32),
        "f_spa": rng.standard_normal((B, 4, D), dtype=np.float32),
        "W": (rng.standard_normal((2 * D, D), dtype=np.float32) * 0.02),
        "b": np.zeros((D,), dtype=np.float32),
    }
    print(kernel(**inputs))


# revision 25
# speedup vs baseline: 1.0332x; 1.0259x over previous
"""Trainium2 Bass kernel for nn_ModalityConsisLoss (8 NeuronCores, data-parallel).

Reference computation:
    v_spa/v_seq = concat([f[:,a,:], f[:,2,:]], -1) @ W + b   for a in (0,1,3)  -> [3B, D]
    z = normalize_rows(concat([v_spa, v_seq]))               -> [6B, D]
    sim = z @ z.T ;  pos = diag pairs (i, i+3B)
    loss = sum(-pos/T) + sum(log(rowsum(exp(sim/T)) - diag)) / (6B)

Strategy (data-parallel over B):
  Each core owns B/8 = 256 batch rows -> 1536 of the 12288 z-rows
  (rows of both modalities for its batch slice, so pos pairs stay local).
  Per core, on device, per modality half (spa then seq):
    - load f shard, PE-transpose -> fT, projection matmuls -> vT half
    - column norms via ones-matmul; r = 16 * rsqrt(ssq) via exp/ln
    - zT_half = fp8_e4m3(vT * r)  [512, 768]  (x16 scaling keeps fp8 in
      normal range; folded back via the exp() scale and the pos term)
    - AllGather the half (so the spa gather overlaps the seq prologue,
      and the seq gather overlaps the first sim tiles)
  sim tiles: DoubleRow fp8 matmuls (K=256 per instruction) of
  zT_local.T @ zT_all with fused exp(sim/(T*256)) + row-sum on ACT.
  denom = rowsum - e^2 ; partial loss = sum(log denom) - (2/T)*sum(pos).
  Host sums the 8 partial scalars (the trivial all-reduce of the loss).
"""
import sys
from contextlib import ExitStack

sys.path.insert(0, "/opt/trn_rl_repo")

import numpy as np

import concourse.bass as bass
import concourse.mybir as mybir
import concourse.tile as tile
from concourse import bacc
from concourse import bass_utils
from concourse.masks import make_identity

F32 = mybir.dt.float32
BF16 = mybir.dt.bfloat16
FP8 = mybir.dt.float8e4
AF = mybir.ActivationFunctionType
ALU = mybir.AluOpType
DR = mybir.MatmulPerfMode.DoubleRow

N_CORES = 8
B = 2048
BL = B // N_CORES          # 256 local batch rows
D = 512
KB = D // 128              # 4 d blocks of 128
HROWS = 3 * BL             # 768 rows per modality half
LROWS = 2 * HROWS          # 1536 local z-rows (spa 768 | seq 768)
R = N_CORES * LROWS        # 12288 total rows
HALL = N_CORES * HROWS     # 6144 gathered columns per half
IB = LROWS // 128          # 12 row blocks of 128 per core
SIMW = 1536                # sim chunk width (3 PSUM banks, one ACT op)
CC = HALL // SIMW          # 3 sim column chunks per half
LH = (0, 1, 3)             # left heads of the pairs (x, 2)
TEMP = 0.5
ZSCALE = 16.0              # fp8 z scaling
ESCALE = (1.0 / TEMP) / (ZSCALE * ZSCALE)
POS_COEF = (-2.0 / TEMP) / (ZSCALE * ZSCALE)
E2 = float(np.exp(2.0))    # diagonal term exp(2 * ||z||^2), ||z|| == 1
INV_COUNT = 1.0 / R        # final 1/(2*half)


def _body(ctx, nc, tc, f_aps, w_ap, b_ap, out_ap):
    const_pool = ctx.enter_context(tc.tile_pool(name="const", bufs=1))
    small_pool = ctx.enter_context(tc.tile_pool(name="small", bufs=1))
    vt_pool = ctx.enter_context(tc.tile_pool(name="vt", bufs=1))
    dram_pool = ctx.enter_context(tc.tile_pool(name="dram", bufs=1,
                                               space="DRAM"))
    big_pool = ctx.enter_context(tc.tile_pool(name="big", bufs=1))

    ident = const_pool.tile([128, 128], F32)
    make_identity(nc, ident[:])
    ones_col = const_pool.tile([128, 1], F32)
    nc.vector.memset(ones_col[:], 1.0)
    ones_row = const_pool.tile([1, 128], F32)
    nc.vector.memset(ones_row[:], 1.0)
    neg_e2 = const_pool.tile([128, 1], F32)
    nc.vector.memset(neg_e2[:], -E2)
    ln_zs = const_pool.tile([1, 1], F32)
    nc.vector.memset(ln_zs[:], float(np.log(ZSCALE)))

    # b columns: [128, 4] (per d_out block)
    b_col = const_pool.tile([128, 4], F32)
    for m in range(KB):
        nc.sync.dma_start(b_col[:, m:m + 1], b_ap[m * 128:(m + 1) * 128])
    w_bf = const_pool.tile([128, 8, D], BF16)

    vT = vt_pool.tile([128, KB, LROWS], F32)       # [d_out(blk,128), rows]
    zT_loc = small_pool.tile([128, KB, LROWS], FP8, tag="zT_loc")
    r_row = small_pool.tile([1, LROWS], F32, tag="r_row")
    zT_all = [None, None]

    with tc.tile_pool(name="fstage", bufs=4) as fst_pool, \
         tc.tile_pool(name="ftrans", bufs=1) as ft_pool, \
         tc.tile_pool(name="sq", bufs=2) as sq_pool, \
         tc.tile_pool(name="ps_t", bufs=2, space="PSUM") as ps_t, \
         tc.tile_pool(name="ps_proj", bufs=2, space="PSUM") as ps_proj, \
         tc.tile_pool(name="ps_s", bufs=2, space="PSUM") as ps_s:

        # f loads first: the transposes (start of the PE critical path)
        # need them; W can land while the first transposes run.
        f_sts = {}
        for mod in range(2):
            for h in range(2):
                f_st = fst_pool.tile([128, 4 * D], F32,
                                     name=f"f_st{mod}{h}", tag="f_st")
                nc.sync.dma_start(
                    f_st[:], f_aps[mod][h * 128:(h + 1) * 128, :, :])
                f_sts[(mod, h)] = f_st

        # W: [1024, 512] f32 -> bf16 [128, 8(kblk), 512(d_out)]
        w_st = fst_pool.tile([128, 8, D], F32, tag="w_st", bufs=1)
        for kb in range(8):
            nc.sync.dma_start(w_st[:, kb, :], w_ap[kb * 128:(kb + 1) * 128, :])
        nc.vector.tensor_copy(w_bf[:], w_st[:])

        for mod in range(2):                   # 0 = spa, 1 = seq
            c0 = mod * HROWS
            # ---- transpose f ----
            fT = ft_pool.tile([128, 4, KB, 2 * 128], BF16, name=f"fT{mod}",
                              tag=f"fT{mod}")
            for h in range(2):                 # halves of 256 local rows
                f_st = f_sts[(mod, h)]
                for a in range(4):
                    for kb in range(KB):
                        pst = ps_t.tile([128, 128], F32, name="pst", tag="pst")
                        nc.tensor.transpose(
                            pst[:],
                            f_st[:, a * D + kb * 128: a * D + (kb + 1) * 128],
                            ident[:])
                        nc.vector.tensor_copy(
                            fT[:, a, kb, h * 128:(h + 1) * 128], pst[:])
            # ---- projection ----
            for pa in range(3):
                for m in range(KB):
                    psv = ps_proj.tile([128, 2 * 128], F32, name="psv",
                                       tag="psv")
                    for kk in range(8):
                        head = LH[pa] if kk < 4 else 2
                        kb = kk % 4
                        nc.tensor.matmul(
                            psv[:],
                            lhsT=w_bf[:, kk, m * 128:(m + 1) * 128],
                            rhs=fT[:, head, kb, :],
                            start=(kk == 0), stop=(kk == 7))
                    col0 = c0 + pa * 256
                    nc.vector.tensor_scalar_add(
                        vT[:, m, col0:col0 + 256], psv[:], b_col[:, m:m + 1])

            # ---- norms: ssq over d for this half's 768 columns ----
            ssq = small_pool.tile([1, HROWS], F32, name=f"ssq{mod}",
                                  tag=f"ssq{mod}")
            for co, cw in ((0, 512), (512, 256)):
                ps_ssq = ps_s.tile([1, 512], F32, name="ps_ssq", tag="ps_s")
                for m in range(KB):
                    sq = sq_pool.tile([128, 512], F32, name="sq", tag="sq")
                    nc.vector.tensor_mul(sq[:, :cw],
                                         vT[:, m, c0 + co:c0 + co + cw],
                                         vT[:, m, c0 + co:c0 + co + cw])
                    nc.tensor.matmul(ps_ssq[:, :cw], lhsT=ones_col[:],
                                     rhs=sq[:, :cw],
                                     start=(m == 0), stop=(m == KB - 1))
                nc.vector.tensor_copy(ssq[:, co:co + cw], ps_ssq[:, :cw])

            # r = ZSCALE / sqrt(ssq) = exp(-0.5*ln(ssq) + ln(ZSCALE))
            lnss = small_pool.tile([1, HROWS], F32, name=f"lnss{mod}",
                                   tag=f"lnss{mod}")
            nc.scalar.activation(lnss[:], ssq[:], AF.Ln)
            nc.scalar.activation(r_row[:, c0:c0 + HROWS], lnss[:], AF.Exp,
                                 scale=-0.5, bias=ln_zs[:])

            # zT_loc half = fp8(vT * r)
            for co, cw in ((0, 512), (512, 256)):
                rb = ps_s.tile([128, 512], F32, name="rb", tag="rb")
                nc.tensor.matmul(rb[:, :cw], lhsT=ones_row[:],
                                 rhs=r_row[:, c0 + co:c0 + co + cw],
                                 start=True, stop=True)
                for m in range(KB):
                    nc.vector.tensor_mul(
                        zT_loc[:, m, c0 + co:c0 + co + cw],
                        vT[:, m, c0 + co:c0 + co + cw], rb[:, :cw])

            # ---- AllGather this half ----
            ag_in = dram_pool.tile([4 * 128, HROWS], FP8, name=f"ag_in{mod}",
                                   tag=f"ag_in{mod}")
            ag_out = dram_pool.tile([N_CORES * 4 * 128, HROWS], FP8,
                                    addr_space="Shared", name=f"ag_out{mod}",
                                    tag=f"ag_out{mod}")
            for m in range(KB):
                nc.sync.dma_start(ag_in[m * 128:(m + 1) * 128, :],
                                  zT_loc[:, m, c0:c0 + HROWS])
            nc.gpsimd.collective_compute(
                "AllGather", ALU.bypass,
                replica_groups=[list(range(N_CORES))],
                ins=[ag_in.opt()], outs=[ag_out.opt()])
            zT_all[mod] = big_pool.tile([128, KB, HALL], FP8,
                                        name=f"zT_all{mod}", tag=f"zTa{mod}")
            for rr in range(N_CORES):
                for m in range(KB):
                    nc.sync.dma_start(
                        zT_all[mod][:, m, rr * HROWS:(rr + 1) * HROWS],
                        ag_out[rr * 512 + m * 128: rr * 512 + (m + 1) * 128, :])

        # ---- pos_i = r_i * r_{i+768} * sum_d vT[d, i] * vT[d, i+768] ----
        pos_raw = small_pool.tile([1, HROWS], F32, tag="pos_raw")
        for co, cw in ((0, 512), (512, 256)):
            ps_pp = ps_s.tile([1, 512], F32, name="ps_pp", tag="ps_s")
            for m in range(KB):
                pp = sq_pool.tile([128, 512], F32, name="pp", tag="sq")
                nc.vector.tensor_mul(pp[:, :cw], vT[:, m, co:co + cw],
                                     vT[:, m, HROWS + co:HROWS + co + cw])
                nc.tensor.matmul(ps_pp[:, :cw], lhsT=ones_col[:],
                                 rhs=pp[:, :cw],
                                 start=(m == 0), stop=(m == KB - 1))
            nc.vector.tensor_copy(pos_raw[:, co:co + cw], ps_pp[:, :cw])
        rrp = small_pool.tile([1, HROWS], F32, tag="rrp")
        nc.vector.tensor_mul(rrp[:], r_row[:, 0:HROWS], r_row[:, HROWS:LROWS])
        pos_row = small_pool.tile([1, HROWS], F32, tag="pos_row")
        nc.vector.tensor_mul(pos_row[:], pos_raw[:], rrp[:])
        pos_sum = small_pool.tile([1, 1], F32, tag="pos_sum")
        nc.vector.tensor_reduce(pos_sum[:], pos_row[:],
                                axis=mybir.AxisListType.X, op=ALU.add)

    # ---------- sim tiles + fused exp/rowsum (DoubleRow fp8) ----------
    # stats col layout: [ib][mod * CC + cc] so the per-ib reduce is a
    # contiguous innermost group of 2*CC.
    stats = small_pool.tile([128, 2 * IB * CC], F32, tag="stats")
    with tc.tile_pool(name="ps_sim", bufs=2, space="PSUM") as ps_sim:
        for mod in range(2):
            for ib in range(IB):
                for cc in range(CC):
                    ps = ps_sim.tile([128, SIMW], F32, name="ps_sim",
                                     tag="ps_sim")
                    for jt in range(SIMW // 512):
                        j0 = cc * SIMW + jt * 512
                        for g in range(2):
                            nc.tensor.matmul(
                                ps[:, jt * 512:(jt + 1) * 512],
                                lhsT=zT_loc[:, 2 * g:2 * g + 2,
                                            ib * 128:(ib + 1) * 128],
                                rhs=zT_all[mod][:, 2 * g:2 * g + 2,
                                                j0:j0 + 512],
                                start=(g == 0), stop=(g == 1),
                                perf_mode=DR)
                    scol = ib * 2 * CC + mod * CC + cc
                    nc.scalar.activation(
                        ps[:], ps[:], AF.Exp, scale=ESCALE,
                        accum_out=stats[:, scol:scol + 1])

    # ---------- final reduction ----------
    with tc.tile_pool(name="ps_fin", bufs=1, space="PSUM") as ps_fin:
        denom = small_pool.tile([128, IB], F32, tag="denom")
        nc.vector.tensor_reduce(
            denom[:], stats.rearrange("p (i x) -> p i x", x=2 * CC),
            axis=mybir.AxisListType.X, op=ALU.add)
        logd = small_pool.tile([128, IB], F32, tag="logd")
        nc.scalar.activation(logd[:], denom[:], AF.Ln, bias=neg_e2[:])
        logsum = small_pool.tile([128, 1], F32, tag="logsum")
        nc.vector.tensor_reduce(logsum[:], logd[:],
                                axis=mybir.AxisListType.X, op=ALU.add)
        fin = ps_fin.tile([1, 1], F32, tag="fin")
        nc.tensor.matmul(fin[:], lhsT=ones_col[:], rhs=logsum[:],
                         start=True, stop=True)
        res = small_pool.tile([1, 1], F32, tag="res")
        # res = (pos_sum * POS_COEF + sum(log denom)) / R
        nc.vector.scalar_tensor_tensor(res[:], pos_sum[:], POS_COEF,
                                       fin[:], op0=ALU.mult, op1=ALU.add)
        nc.vector.tensor_scalar_mul(res[:], res[:], INV_COUNT)
        nc.sync.dma_start(out_ap[:], res[:])


_NC_CACHE = None


def build_nc():
    global _NC_CACHE
    if _NC_CACHE is not None:
        return _NC_CACHE
    nc = bacc.Bacc("TRN2", target_bir_lowering=False, debug=False,
                   num_devices=N_CORES)
    f_spa = nc.dram_tensor("f_spa", [BL, 4, D], F32, kind="ExternalInput").ap()
    f_seq = nc.dram_tensor("f_seq", [BL, 4, D], F32, kind="ExternalInput").ap()
    w_ap = nc.dram_tensor("W", [2 * D, D], F32, kind="ExternalInput").ap()
    b_ap = nc.dram_tensor("b", [D], F32, kind="ExternalInput").ap()
    out_ap = nc.dram_tensor("out", [1, 1], F32, kind="ExternalOutput").ap()
    with tile.TileContext(nc) as tc, ExitStack() as ctx:
        _body(ctx, nc, tc, (f_spa, f_seq), w_ap, b_ap, out_ap)
    nc.compile()
    _NC_CACHE = nc
    return nc


def run(inputs, **kw):
    nc = build_nc()
    f_seq = np.ascontiguousarray(np.asarray(inputs["f_seq"], dtype=np.float32))
    f_spa = np.ascontiguousarray(np.asarray(inputs["f_spa"], dtype=np.float32))
    W = np.ascontiguousarray(np.asarray(inputs["W"], dtype=np.float32))
    b = np.ascontiguousarray(np.asarray(inputs["b"], dtype=np.float32))
    in_maps = []
    for c in range(N_CORES):
        sl = slice(c * BL, (c + 1) * BL)
        in_maps.append({"f_seq": np.ascontiguousarray(f_seq[sl]),
                        "f_spa": np.ascontiguousarray(f_spa[sl]),
                        "W": W, "b": b})
    try:
        res = bass_utils.run_bass_kernel_spmd(
            nc, in_maps, core_ids=list(range(N_CORES)), **kw)
    except Exception:
        # the axon terminal occasionally reports a transient
        # "device unrecoverable" on first attach; one retry clears it
        import time
        time.sleep(15)
        res = bass_utils.run_bass_kernel_spmd(
            nc, in_maps, core_ids=list(range(N_CORES)), **kw)
    total = np.float64(0.0)
    for c in range(N_CORES):
        total += np.float64(res.results[c]["out"][0, 0])
    return np.float32(total), res


def kernel(**inputs) -> np.ndarray:
    loss, _ = run(inputs)
    return np.asarray(loss, dtype=np.float32)


if __name__ == "__main__":
    rng = np.random.default_rng(0)
    inputs = {
        "f_seq": rng.standard_normal((B, 4, D), dtype=np.float32),
        "f_spa": rng.standard_normal((B, 4, D), dtype=np.float32),
        "W": (rng.standard_normal((2 * D, D), dtype=np.float32) * 0.02),
        "b": np.zeros((D,), dtype=np.float32),
    }
    print(kernel(**inputs))


# revision 30
# speedup vs baseline: 1.0552x; 1.0212x over previous
"""Trainium2 Bass kernel for nn_ModalityConsisLoss (8 NeuronCores, data-parallel).

Reference computation:
    v_spa/v_seq = concat([f[:,a,:], f[:,2,:]], -1) @ W + b   for a in (0,1,3)  -> [3B, D]
    z = normalize_rows(concat([v_spa, v_seq]))               -> [6B, D]
    sim = z @ z.T ;  pos = diag pairs (i, i+3B)
    loss = sum(-pos/T) + sum(log(rowsum(exp(sim/T)) - diag)) / (6B)

Strategy (data-parallel over B):
  Each core owns B/8 = 256 batch rows -> 1536 of the 12288 z-rows
  (rows of both modalities for its batch slice, so pos pairs stay local).
  Per core, on device, per modality half (spa then seq):
    - load f shard, PE-transpose -> fT, projection matmuls -> vT half
    - column norms via ones-matmul; r = 16 * rsqrt(ssq) via exp/ln
    - zT_half = fp8_e4m3(vT * r)  [512, 768]  (x16 scaling keeps fp8 in
      normal range; folded back via the exp() scale and the pos term)
    - AllGather the half (so the spa gather overlaps the seq prologue,
      and the seq gather overlaps the first sim tiles)
  sim tiles: DoubleRow fp8 matmuls (K=256 per instruction) of
  zT_local.T @ zT_all with fused exp(sim/(T*256)) + row-sum on ACT.
  denom = rowsum - e^2 ; partial loss = sum(log denom) - (2/T)*sum(pos).
  Host sums the 8 partial scalars (the trivial all-reduce of the loss).
"""
import sys
from contextlib import ExitStack

sys.path.insert(0, "/opt/trn_rl_repo")

import numpy as np

import concourse.bass as bass
import concourse.mybir as mybir
import concourse.tile as tile
from concourse import bacc
from concourse import bass_utils
from concourse.masks import make_identity

F32 = mybir.dt.float32
BF16 = mybir.dt.bfloat16
FP8 = mybir.dt.float8e4
AF = mybir.ActivationFunctionType
ALU = mybir.AluOpType
DR = mybir.MatmulPerfMode.DoubleRow

N_CORES = 8
B = 2048
BL = B // N_CORES          # 256 local batch rows
D = 512
KB = D // 128              # 4 d blocks of 128
HROWS = 3 * BL             # 768 rows per modality half
LROWS = 2 * HROWS          # 1536 local z-rows (spa 768 | seq 768)
R = N_CORES * LROWS        # 12288 total rows
HALL = N_CORES * HROWS     # 6144 gathered columns per half
IB = LROWS // 128          # 12 row blocks of 128 per core
SIMW = 1536                # sim chunk width (3 PSUM banks, one ACT op)
CC = HALL // SIMW          # 3 sim column chunks per half
LH = (0, 1, 3)             # left heads of the pairs (x, 2)
TEMP = 0.5
ZSCALE = 16.0              # fp8 z scaling
ESCALE = (1.0 / TEMP) / (ZSCALE * ZSCALE)
POS_COEF = (-2.0 / TEMP) / (ZSCALE * ZSCALE)
E2 = float(np.exp(2.0))    # diagonal term exp(2 * ||z||^2), ||z|| == 1
INV_COUNT = 1.0 / R        # final 1/(2*half)


def _body(ctx, nc, tc, f_aps, w_ap, b_ap, out_ap):
    const_pool = ctx.enter_context(tc.tile_pool(name="const", bufs=1))
    small_pool = ctx.enter_context(tc.tile_pool(name="small", bufs=1))
    vt_pool = ctx.enter_context(tc.tile_pool(name="vt", bufs=1))
    dram_pool = ctx.enter_context(tc.tile_pool(name="dram", bufs=1,
                                               space="DRAM"))
    big_pool = ctx.enter_context(tc.tile_pool(name="big", bufs=1))

    ident = const_pool.tile([128, 128], F32)
    make_identity(nc, ident[:])
    ones_col = const_pool.tile([128, 1], F32)
    nc.vector.memset(ones_col[:], 1.0)
    ones_row = const_pool.tile([1, 128], F32)
    nc.vector.memset(ones_row[:], 1.0)
    neg_e2 = const_pool.tile([128, 1], F32)
    nc.vector.memset(neg_e2[:], -E2)
    ln_zs = const_pool.tile([1, 1], F32)
    nc.vector.memset(ln_zs[:], float(np.log(ZSCALE)))

    # b columns: [128, 4] (per d_out block)
    b_col = const_pool.tile([128, 4], F32)
    for m in range(KB):
        nc.sync.dma_start(b_col[:, m:m + 1], b_ap[m * 128:(m + 1) * 128])
    w_bf = const_pool.tile([128, 8, D], BF16)

    vT = vt_pool.tile([128, KB, LROWS], F32)       # [d_out(blk,128), rows]
    zT_loc = small_pool.tile([128, KB, LROWS], FP8, tag="zT_loc")
    r_row = small_pool.tile([1, LROWS], F32, tag="r_row")
    zT_all = [None, None]

    with tc.tile_pool(name="fstage", bufs=4) as fst_pool, \
         tc.tile_pool(name="ftrans", bufs=1) as ft_pool, \
         tc.tile_pool(name="sq", bufs=2) as sq_pool, \
         tc.tile_pool(name="ps_t", bufs=2, space="PSUM") as ps_t, \
         tc.tile_pool(name="ps_proj", bufs=2, space="PSUM") as ps_proj, \
         tc.tile_pool(name="ps_s", bufs=2, space="PSUM") as ps_s:

        # f loads first: the transposes (start of the PE critical path)
        # need them; W can land while the first transposes run.
        f_sts = {}
        for mod in range(2):
            for h in range(2):
                f_st = fst_pool.tile([128, 4 * D], F32,
                                     name=f"f_st{mod}{h}", tag="f_st")
                nc.sync.dma_start(
                    f_st[:], f_aps[mod][h * 128:(h + 1) * 128, :, :])
                f_sts[(mod, h)] = f_st

        # W: [1024, 512] f32 -> bf16 [128, 8(kblk), 512(d_out)]
        w_st = fst_pool.tile([128, 8, D], F32, tag="w_st", bufs=1)
        for kb in range(8):
            nc.sync.dma_start(w_st[:, kb, :], w_ap[kb * 128:(kb + 1) * 128, :])
        nc.vector.tensor_copy(w_bf[:], w_st[:])

        for mod in range(2):                   # 0 = spa, 1 = seq
            c0 = mod * HROWS
            # ---- transpose f ----
            fT = ft_pool.tile([128, 4, KB, 2 * 128], BF16, name=f"fT{mod}",
                              tag=f"fT{mod}")
            for h in range(2):                 # halves of 256 local rows
                f_st = f_sts[(mod, h)]
                for a in range(4):
                    for kb in range(KB):
                        pst = ps_t.tile([128, 128], F32, name="pst", tag="pst")
                        nc.tensor.transpose(
                            pst[:],
                            f_st[:, a * D + kb * 128: a * D + (kb + 1) * 128],
                            ident[:])
                        nc.vector.tensor_copy(
                            fT[:, a, kb, h * 128:(h + 1) * 128], pst[:])
            # ---- projection ----
            for pa in range(3):
                for m in range(KB):
                    psv = ps_proj.tile([128, 2 * 128], F32, name="psv",
                                       tag="psv")
                    for kk in range(8):
                        head = LH[pa] if kk < 4 else 2
                        kb = kk % 4
                        nc.tensor.matmul(
                            psv[:],
                            lhsT=w_bf[:, kk, m * 128:(m + 1) * 128],
                            rhs=fT[:, head, kb, :],
                            start=(kk == 0), stop=(kk == 7))
                    col0 = c0 + pa * 256
                    nc.vector.tensor_scalar_add(
                        vT[:, m, col0:col0 + 256], psv[:], b_col[:, m:m + 1])

            # ---- norms: ssq over d for this half's 768 columns ----
            ssq = small_pool.tile([1, HROWS], F32, name=f"ssq{mod}",
                                  tag=f"ssq{mod}")
            for co, cw in ((0, 512), (512, 256)):
                ps_ssq = ps_s.tile([1, 512], F32, name="ps_ssq", tag="ps_s")
                for m in range(KB):
                    sq = sq_pool.tile([128, 512], F32, name="sq", tag="sq")
                    nc.vector.tensor_mul(sq[:, :cw],
                                         vT[:, m, c0 + co:c0 + co + cw],
                                         vT[:, m, c0 + co:c0 + co + cw])
                    nc.tensor.matmul(ps_ssq[:, :cw], lhsT=ones_col[:],
                                     rhs=sq[:, :cw],
                                     start=(m == 0), stop=(m == KB - 1))
                nc.vector.tensor_copy(ssq[:, co:co + cw], ps_ssq[:, :cw])

            # r = ZSCALE / sqrt(ssq) = exp(-0.5*ln(ssq) + ln(ZSCALE))
            lnss = small_pool.tile([1, HROWS], F32, name=f"lnss{mod}",
                                   tag=f"lnss{mod}")
            nc.scalar.activation(lnss[:], ssq[:], AF.Ln)
            nc.scalar.activation(r_row[:, c0:c0 + HROWS], lnss[:], AF.Exp,
                                 scale=-0.5, bias=ln_zs[:])

            # zT_loc half = fp8(vT * r)
            for co, cw in ((0, 512), (512, 256)):
                rb = ps_s.tile([128, 512], F32, name="rb", tag="rb")
                nc.tensor.matmul(rb[:, :cw], lhsT=ones_row[:],
                                 rhs=r_row[:, c0 + co:c0 + co + cw],
                                 start=True, stop=True)
                for m in range(KB):
                    nc.vector.tensor_mul(
                        zT_loc[:, m, c0 + co:c0 + co + cw],
                        vT[:, m, c0 + co:c0 + co + cw], rb[:, :cw])

            # ---- AllGather this half ----
            ag_in = dram_pool.tile([4 * 128, HROWS], FP8, name=f"ag_in{mod}",
                                   tag=f"ag_in{mod}")
            ag_out = dram_pool.tile([N_CORES * 4 * 128, HROWS], FP8,
                                    addr_space="Shared", name=f"ag_out{mod}",
                                    tag=f"ag_out{mod}")
            for m in range(KB):
                nc.sync.dma_start(ag_in[m * 128:(m + 1) * 128, :],
                                  zT_loc[:, m, c0:c0 + HROWS])
            nc.gpsimd.collective_compute(
                "AllGather", ALU.bypass,
                replica_groups=[list(range(N_CORES))],
                ins=[ag_in.opt()], outs=[ag_out.opt()])
            zT_all[mod] = big_pool.tile([128, KB, HALL], FP8,
                                        name=f"zT_all{mod}", tag=f"zTa{mod}")
            for rr in range(N_CORES):
                for m in range(KB):
                    nc.sync.dma_start(
                        zT_all[mod][:, m, rr * HROWS:(rr + 1) * HROWS],
                        ag_out[rr * 512 + m * 128: rr * 512 + (m + 1) * 128, :])

        # ---- pos_i = r_i * r_{i+768} * sum_d vT[d, i] * vT[d, i+768] ----
        pos_raw = small_pool.tile([1, HROWS], F32, tag="pos_raw")
        for co, cw in ((0, 512), (512, 256)):
            ps_pp = ps_s.tile([1, 512], F32, name="ps_pp", tag="ps_s")
            for m in range(KB):
                pp = sq_pool.tile([128, 512], F32, name="pp", tag="sq")
                nc.vector.tensor_mul(pp[:, :cw], vT[:, m, co:co + cw],
                                     vT[:, m, HROWS + co:HROWS + co + cw])
                nc.tensor.matmul(ps_pp[:, :cw], lhsT=ones_col[:],
                                 rhs=pp[:, :cw],
                                 start=(m == 0), stop=(m == KB - 1))
            nc.vector.tensor_copy(pos_raw[:, co:co + cw], ps_pp[:, :cw])
        rrp = small_pool.tile([1, HROWS], F32, tag="rrp")
        nc.vector.tensor_mul(rrp[:], r_row[:, 0:HROWS], r_row[:, HROWS:LROWS])
        pos_row = small_pool.tile([1, HROWS], F32, tag="pos_row")
        nc.vector.tensor_mul(pos_row[:], pos_raw[:], rrp[:])
        pos_sum = small_pool.tile([1, 1], F32, tag="pos_sum")
        nc.vector.tensor_reduce(pos_sum[:], pos_row[:],
                                axis=mybir.AxisListType.X, op=ALU.add)

    # ---------- sim tiles + fused exp/rowsum (DoubleRow fp8) ----------
    # The sim matrix is symmetric in its modality blocks:
    #   [ A  C ]   A = spa x spa, B = seq x seq, C = spa x seq.
    #   [ C' B ]
    # We never compute C': its row sums (the seq rows' spa-column denom
    # contributions) are recovered as COLUMN sums of C via ones-matmuls,
    # then summed across cores with a ReduceScatter, whose shard-per-rank
    # output is exactly this core's seq rows (SPMD-uniform by construction).
    # Cuts the exp work (the saturated ACT engine) and the sim matmuls by 25%.
    #
    # stats col layout: [ib][mod * CC + cc]; the mod0 columns of seq row
    # blocks (the dropped C' chunks) stay zero.
    HIB = IB // 2
    stats = small_pool.tile([128, 2 * IB * CC], F32, tag="stats")
    nc.vector.memset(stats[:], 0.0)
    colacc = small_pool.tile([1, HALL], F32, tag="colacc")
    nc.vector.memset(colacc[:], 0.0)
    ones_col_b = const_pool.tile([128, 1], BF16)
    nc.vector.memset(ones_col_b[:], 1.0)
    colden = small_pool.tile([128, HIB], F32, tag="colden")

    def sim_chunk(ps_sim, mod, ib, cc):
        ps = ps_sim.tile([128, SIMW], F32, name="ps_sim", tag="ps_sim")
        for jt in range(SIMW // 512):
            j0 = cc * SIMW + jt * 512
            for g in range(2):
                nc.tensor.matmul(
                    ps[:, jt * 512:(jt + 1) * 512],
                    lhsT=zT_loc[:, 2 * g:2 * g + 2, ib * 128:(ib + 1) * 128],
                    rhs=zT_all[mod][:, 2 * g:2 * g + 2, j0:j0 + 512],
                    start=(g == 0), stop=(g == 1), perf_mode=DR)
        return ps

    with tc.tile_pool(name="ps_sim", bufs=2, space="PSUM") as ps_sim, \
         tc.tile_pool(name="ps_cs", bufs=2, space="PSUM") as ps_cs, \
         tc.tile_pool(name="esb", bufs=3) as esb_pool:
        # phase A: spa rows x spa cols (row sums only)
        for ib in range(HIB):
            for cc in range(CC):
                ps = sim_chunk(ps_sim, 0, ib, cc)
                scol = ib * 2 * CC + cc
                nc.scalar.activation(ps[:], ps[:], AF.Exp, scale=ESCALE,
                                     accum_out=stats[:, scol:scol + 1])
        # phase C: spa rows x seq cols (row sums + column sums)
        for ib in range(HIB):
            for cc in range(CC):
                ps = sim_chunk(ps_sim, 1, ib, cc)
                scol = ib * 2 * CC + CC + cc
                e_sb = esb_pool.tile([128, SIMW], BF16, name="e_sb",
                                     tag="e_sb")
                nc.scalar.activation(e_sb[:], ps[:], AF.Exp, scale=ESCALE,
                                     accum_out=stats[:, scol:scol + 1])
                for jt in range(SIMW // 512):
                    pc = ps_cs.tile([1, 512], F32, name="pc", tag="pc")
                    nc.tensor.matmul(pc[:], lhsT=ones_col_b[:],
                                     rhs=e_sb[:, jt * 512:(jt + 1) * 512],
                                     start=True, stop=True)
                    sl = slice(cc * SIMW + jt * 512, cc * SIMW + (jt + 1) * 512)
                    nc.vector.tensor_add(colacc[:, sl], colacc[:, sl], pc[:])
        # ReduceScatter the seq-row column contributions: rank r's output
        # shard is rows [r*768, (r+1)*768) = exactly our local seq rows.
        rs_in = dram_pool.tile([HALL], F32, tag="rs_in")
        rs_out = dram_pool.tile([HROWS], F32, tag="rs_out")
        nc.sync.dma_start(rs_in[:], colacc[:])
        nc.gpsimd.collective_compute(
            "ReduceScatter", ALU.add,
            replica_groups=[list(range(N_CORES))],
            ins=[rs_in.opt()], outs=[rs_out.opt()])
        for j in range(HIB):
            nc.sync.dma_start(colden[:, j:j + 1],
                              rs_out[j * 128:(j + 1) * 128])
        # phase B: seq rows x seq cols (row sums only)
        for ib in range(HIB, IB):
            for cc in range(CC):
                ps = sim_chunk(ps_sim, 1, ib, cc)
                scol = ib * 2 * CC + CC + cc
                nc.scalar.activation(ps[:], ps[:], AF.Exp, scale=ESCALE,
                                     accum_out=stats[:, scol:scol + 1])

    # ---------- final reduction ----------
    with tc.tile_pool(name="ps_fin", bufs=1, space="PSUM") as ps_fin:
        denom = small_pool.tile([128, IB], F32, tag="denom")
        nc.vector.tensor_reduce(
            denom[:], stats.rearrange("p (i x) -> p i x", x=2 * CC),
            axis=mybir.AxisListType.X, op=ALU.add)
        # seq rows: add the ReduceScattered spa-column contributions
        nc.vector.tensor_add(denom[:, HIB:IB], denom[:, HIB:IB], colden[:])
        logd = small_pool.tile([128, IB], F32, tag="logd")
        nc.scalar.activation(logd[:], denom[:], AF.Ln, bias=neg_e2[:])
        logsum = small_pool.tile([128, 1], F32, tag="logsum")
        nc.vector.tensor_reduce(logsum[:], logd[:],
                                axis=mybir.AxisListType.X, op=ALU.add)
        fin = ps_fin.tile([1, 1], F32, tag="fin")
        nc.tensor.matmul(fin[:], lhsT=ones_col[:], rhs=logsum[:],
                         start=True, stop=True)
        res = small_pool.tile([1, 1], F32, tag="res")
        # res = (pos_sum * POS_COEF + sum(log denom)) / R
        nc.vector.scalar_tensor_tensor(res[:], pos_sum[:], POS_COEF,
                                       fin[:], op0=ALU.mult, op1=ALU.add)
        nc.vector.tensor_scalar_mul(res[:], res[:], INV_COUNT)
        nc.sync.dma_start(out_ap[:], res[:])


_NC_CACHE = None


def build_nc():
    global _NC_CACHE
    if _NC_CACHE is not None:
        return _NC_CACHE
    nc = bacc.Bacc("TRN2", target_bir_lowering=False, debug=False,
                   num_devices=N_CORES)
    f_spa = nc.dram_tensor("f_spa", [BL, 4, D], F32, kind="ExternalInput").ap()
    f_seq = nc.dram_tensor("f_seq", [BL, 4, D], F32, kind="ExternalInput").ap()
    w_ap = nc.dram_tensor("W", [2 * D, D], F32, kind="ExternalInput").ap()
    b_ap = nc.dram_tensor("b", [D], F32, kind="ExternalInput").ap()
    out_ap = nc.dram_tensor("out", [1, 1], F32, kind="ExternalOutput").ap()
    with tile.TileContext(nc) as tc, ExitStack() as ctx:
        _body(ctx, nc, tc, (f_spa, f_seq), w_ap, b_ap, out_ap)
    nc.compile()
    _NC_CACHE = nc
    return nc


def run(inputs, **kw):
    nc = build_nc()
    f_seq = np.ascontiguousarray(np.asarray(inputs["f_seq"], dtype=np.float32))
    f_spa = np.ascontiguousarray(np.asarray(inputs["f_spa"], dtype=np.float32))
    W = np.ascontiguousarray(np.asarray(inputs["W"], dtype=np.float32))
    b = np.ascontiguousarray(np.asarray(inputs["b"], dtype=np.float32))
    in_maps = []
    for c in range(N_CORES):
        sl = slice(c * BL, (c + 1) * BL)
        in_maps.append({"f_seq": np.ascontiguousarray(f_seq[sl]),
                        "f_spa": np.ascontiguousarray(f_spa[sl]),
                        "W": W, "b": b})
    try:
        res = bass_utils.run_bass_kernel_spmd(
            nc, in_maps, core_ids=list(range(N_CORES)), **kw)
    except Exception:
        # the axon terminal occasionally reports a transient
        # "device unrecoverable" on first attach; one retry clears it
        import time
        time.sleep(15)
        res = bass_utils.run_bass_kernel_spmd(
            nc, in_maps, core_ids=list(range(N_CORES)), **kw)
    total = np.float64(0.0)
    for c in range(N_CORES):
        total += np.float64(res.results[c]["out"][0, 0])
    return np.float32(total), res


def kernel(**inputs) -> np.ndarray:
    loss, _ = run(inputs)
    return np.asarray(loss, dtype=np.float32)


if __name__ == "__main__":
    rng = np.random.default_rng(0)
    inputs = {
        "f_seq": rng.standard_normal((B, 4, D), dtype=np.float32),
        "f_spa": rng.standard_normal((B, 4, D), dtype=np.float32),
        "W": (rng.standard_normal((2 * D, D), dtype=np.float32) * 0.02),
        "b": np.zeros((D,), dtype=np.float32),
    }
    print(kernel(**inputs))
